# revision 5
# baseline (speedup 1.0000x reference)
"""Trainium2 Bass kernel for y = x @ W^T + b  (4096x4096 @ 4096x4096 + 4096).

Sharding: data-parallel over batch, R=8 groups. Core r computes
yT_r = W @ x_r^T + b[:, None]  ([4096, 512], output transposed) and the
host reassembles y. No collectives.

All layout work happens on the host: x and W are transposed, tiled to
the exact SBUF layout, and cast to bf16 in numpy. The device kernel is
nothing but back-to-back bf16 matmuls (fp32 PSUM accumulate).

v4 schedule. Measured queue behavior (v1-v3 ntff traces): the gpsimd
SWDGE ring takes ~2/3 of the ~340 GB/s fabric when contending; the two
HWDGE queues (sync/scalar) split the rest. The PE needs W at 148 GB/s
sustained, x only needs 4MB once. So:
  - W streams on gpsimd (the aggressive queue) as 256KB k-group pieces
    in exact PE-consumption order, o5 trickled behind, o6 prefetched;
    steady slabs o7+ also on gpsimd (solo it does ~240 GB/s).
  - The first W pieces (k0-7 x o0..o4, 1.25MB) go on sync, which is
    fast (~240 GB/s) before the gpsimd ring ramps up (~10.7us).
  - x rides both HWDGE queues as alternating 2-k-tile chunks; a 5-deep
    o-tile prologue (k-major) tolerates the slower x cadence
    (~0.9-1.1us/k-tile vs 1.08us of matmul work per k-tile).
  - A junk-matmul burst (memset-gated) keeps the PE busy from ~7us
    (after the fixed ~6.6us framework preamble) so the HAM clock gate
    releases before real matmuls start ~8.9us.
  - Steady state per o-tile: 32 k-matmuls (N=512) into one PSUM bank,
    ScalarE eviction fused with bias add, out DMA on scalar.
  - Tail: the final o-tile's output DMA is split into partition halves
    on sync+scalar (2KB rows, line rate, in parallel).

PE roofline: 1024 MM x 512 cols @ ~2.4 GHz + issue = ~221 us.
"""

import os
import sys

for _p in ("/opt/trn_rl_repo", "/opt/pypackages"):
    if _p not in sys.path and os.path.isdir(_p):
        sys.path.append(_p)

import numpy as np
import ml_dtypes

import concourse.bass as bass
import concourse.tile as tile
from concourse import bacc, mybir
from concourse.bass_utils import run_bass_kernel_spmd

N_CORES = 8
R = 8                          # batch groups
BATCH = 4096
IN_F = 4096
OUT_F = 4096
P = 128
BR = BATCH // R                # 512 batch rows per core
KT = IN_F // P                 # 32 contraction tiles
OT = OUT_F // P                # 32 output-feature tiles per core

NPRE = 5                       # o-tiles in the k-major prologue
NTRICKLE = 1                   # o5 trickled behind the prologue groups
NPREF = 1                      # o6 prefetched as a full slab

_F32 = mybir.dt.float32
_BF16 = mybir.dt.bfloat16
_BF16_NP = ml_dtypes.bfloat16

_compiled_nc = None


def _build():
    nc = bacc.Bacc("TRN2", target_bir_lowering=False, debug=False,
                   num_devices=N_CORES)

    # Host-pretiled layouts (see _prep_inputs):
    #   xt[p, it*BR + b]            = x_r[b, it*128 + p]            (bf16)
    #   wt[ot*128 + p, it*128 + o2] = w[ot*128 + o2, it*128 + p]    (bf16)
    #   bias_t[p, ot]               = b[ot*128 + p]                 (f32)
    xt = nc.dram_tensor("xt", [P, KT * BR], _BF16, kind="ExternalInput")
    wt = nc.dram_tensor("wt", [OT * P, KT * P], _BF16, kind="ExternalInput")
    bias = nc.dram_tensor("bias", [P, OT], _F32, kind="ExternalInput")
    out = nc.dram_tensor("out", [OUT_F, BR], _F32, kind="ExternalOutput")

    with tile.TileContext(nc) as tc:
        with tc.tile_pool(name="const", bufs=1) as const, \
             tc.tile_pool(name="wslab", bufs=7) as wpool, \
             tc.tile_pool(name="psum", bufs=7, space="PSUM") as pspool, \
             tc.tile_pool(name="yout", bufs=3) as ypool:

            bias_sb = const.tile([P, OT], _F32)
            nc.scalar.dma_start(out=bias_sb[:], in_=bias[:, :])

            # PE warm-up fuel with no DMA dependency.
            dummy = const.tile([P, P + BR], _BF16)
            nc.vector.memset(dummy[:], 1.0)

            ps_junk = pspool.tile([P, BR], _F32, name="psjunk", tag="ps")

            def junk_mms(n):
                for i in range(n):
                    nc.tensor.matmul(ps_junk[:], lhsT=dummy[:, 0:P],
                                     rhs=dummy[:, P:P + BR],
                                     start=(i == 0), stop=(i == n - 1))

            # ---- x: alternating 2-k-tile chunks on the two HWDGE queues
            x_sb = const.tile([P, KT * BR], _BF16)

            def xdma(eng, it0, nit):
                eng.dma_start(out=x_sb[:, it0 * BR:(it0 + nit) * BR],
                              in_=xt[:, it0 * BR:(it0 + nit) * BR])

            # scalar: bias, k0, k1, then odd 2-tile chunks
            xdma(nc.scalar, 0, 1)
            xdma(nc.scalar, 1, 1)

            # ---- W tiles for prologue (o0..o4), o5 trickle, o6 prefetch
            NW = NPRE + NTRICKLE + NPREF
            w_pre = [wpool.tile([P, KT * P], _BF16, name=f"w{ot}", tag="w")
                     for ot in range(NW)]

            def wdma(eng, ot, k0, nk):
                eng.dma_start(
                    out=w_pre[ot][:, k0 * P:(k0 + nk) * P],
                    in_=wt[ot * P:(ot + 1) * P, k0 * P:(k0 + nk) * P])

            # sync: first k-groups (k0-7) for all prologue o-tiles (fast
            # before the gpsimd ring ramps), then its share of x chunks.
            for ot in range(NPRE):
                wdma(nc.sync, ot, 0, 8)
            for c in range(2, KT, 4):            # sync x chunks: k2-3, k6-7, ...
                xdma(nc.sync, c, 2)
            for c in range(4, KT, 4):            # scalar x chunks: k4-5, k8-9, ...
                xdma(nc.scalar, c, 2)

            # gpsimd: the W bulk, k-group-major in PE consumption order
            for gi, k0 in enumerate((8, 16, 24)):
                for ot in range(NPRE):
                    wdma(nc.gpsimd, ot, k0, 8)
                wdma(nc.gpsimd, NPRE, gi * 8, 8)     # o5 trickle
            wdma(nc.gpsimd, NPRE, 24, 8)
            wdma(nc.gpsimd, NPRE + 1, 0, KT)         # o6 full slab

            ps_pre = [pspool.tile([P, BR], _F32, name=f"psp{ot}", tag="ps")
                      for ot in range(NPRE + 1)]

            def mm(ot, it, ps=None, w=None):
                ps = ps if ps is not None else ps_pre[ot]
                w = w if w is not None else w_pre[ot]
                nc.tensor.matmul(
                    ps[:],
                    lhsT=w[:, it * P:(it + 1) * P],
                    rhs=x_sb[:, it * BR:(it + 1) * BR],
                    start=(it == 0), stop=(it == KT - 1))

            def evict(ot, ps, split=False):
                y_sb = ypool.tile([P, BR], _F32, name=f"y{ot}", tag="y")
                nc.scalar.activation(y_sb[:], ps[:],
                                     mybir.ActivationFunctionType.Identity,
                                     bias=bias_sb[:, ot:ot + 1])
                if split:
                    HP = P // 2
                    nc.sync.dma_start(out=out[ot * P:ot * P + HP, :],
                                      in_=y_sb[0:HP, :])
                    nc.scalar.dma_start(out=out[ot * P + HP:(ot + 1) * P, :],
                                        in_=y_sb[HP:P, :])
                else:
                    nc.scalar.dma_start(out=out[ot * P:(ot + 1) * P, :],
                                        in_=y_sb[:])

            # ---- prologue: k-major over o0..o4, o3/o4 join after their
            # sync W pieces land (~12.5/13.6us)
            junk_mms(4)
            for it in (0, 1):
                for ot in range(3):
                    mm(ot, it)
            junk_mms(4)
            for it in (0, 1):
                mm(3, it)
            for it in (0, 1):
                mm(4, it)
            for it in range(2, KT):
                for ot in range(NPRE):
                    mm(ot, it)
            for ot in range(NPRE):
                evict(ot, ps_pre[ot])

            # o5 (trickled W), o6 (prefetched slab)
            for it in range(KT):
                mm(NPRE, it)
            evict(NPRE, ps_pre[NPRE])

            ps6 = pspool.tile([P, BR], _F32, name="ps6", tag="ps")
            for it in range(KT):
                mm(NPRE + 1, it, ps=ps6, w=w_pre[NPRE + 1])
            evict(NPRE + 1, ps6)

            # ---- steady state over o-tiles 7..31, W slabs on gpsimd
            for ot in range(NW, OT):
                w_sb = wpool.tile([P, KT * P], _BF16, name=f"w{ot}", tag="w")
                nc.gpsimd.dma_start(out=w_sb[:],
                                    in_=wt[ot * P:(ot + 1) * P, :])
                ps = pspool.tile([P, BR], _F32, name=f"ps{ot}", tag="ps")
                for it in range(KT):
                    mm(ot, it, ps=ps, w=w_sb)
                evict(ot, ps, split=(ot == OT - 1))

    nc.compile()
    return nc


def _get_nc():
    global _compiled_nc
    if _compiled_nc is None:
        _compiled_nc = _build()
    return _compiled_nc


def _prep_inputs(inputs):
    x = np.ascontiguousarray(np.asarray(inputs["x"], dtype=np.float32))
    w = np.ascontiguousarray(np.asarray(inputs["weight"], dtype=np.float32))
    b = np.ascontiguousarray(np.asarray(inputs["bias"], dtype=np.float32))

    # x tiles per batch group r: [p, it*BR + b] = x_r[b, it*128 + p]
    xts = []
    for r in range(R):
        xs = x[r * BR:(r + 1) * BR, :]                      # [BR, IN_F]
        xt = xs.T.reshape(KT, P, BR).transpose(1, 0, 2)     # [P, KT, BR]
        xts.append(np.ascontiguousarray(
            xt.astype(_BF16_NP).reshape(P, KT * BR)))

    # W tiles: [ot*128 + p, it*128 + o2] = w[ot*128 + o2, it*128 + p]
    wtt = w.T.reshape(KT, P, OT, P).transpose(2, 1, 0, 3)   # [OT,P,KT,P]
    wt = np.ascontiguousarray(wtt.astype(_BF16_NP).reshape(OT * P, KT * P))
    bias_t = np.ascontiguousarray(b.reshape(OT, P).T)       # [P, OT]

    return [{"xt": xts[r], "wt": wt, "bias": bias_t} for r in range(R)]


def _run(inputs, trace=False, trace_cores=None):
    nc = _get_nc()
    in_maps = _prep_inputs(inputs)
    res = run_bass_kernel_spmd(nc, in_maps, core_ids=list(range(N_CORES)),
                               trace=trace, trace_cores=trace_cores)
    y = np.empty((BATCH, OUT_F), dtype=np.float32)
    for r in range(R):
        y[r * BR:(r + 1) * BR, :] = res.results[r]["out"].T
    return y, res


def kernel(**inputs):
    y, _ = _run(inputs)
    return y


# revision 6
# speedup vs baseline: 1.0014x; 1.0014x over previous
"""Trainium2 Bass kernel for y = x @ W^T + b  (4096x4096 @ 4096x4096 + 4096).

Sharding: data-parallel over batch, R=8 groups. Core r computes
yT_r = W @ x_r^T + b[:, None]  ([4096, 512], output transposed) and the
host reassembles y. No collectives.

All layout work happens on the host: x and W are transposed, tiled to
the exact SBUF layout, and cast to bf16 in numpy. The device kernel is
nothing but back-to-back bf16 matmuls (fp32 PSUM accumulate).

v5 = the v1 topology (x on the gpsimd ring in graduated k-order chunks
at ~238 GB/s, W as full 1MB slabs on the sync HWDGE queue, bias + outs
on scalar) with trace-driven deltas:
  - Junk warm-up matmuls gated only on a local memset, so the PE is
    busy from ~7us (right after the ~6.6us framework preamble) and the
    HAM clock gate releases at ~10.5us; v1 gated them on the bias DMA
    and its first real matmuls ran at the throttled 1.2 GHz clock.
  - Three prologue o-tiles with STAGGERED joins matched to measured
    slab arrival (~12.4 / ~18 / ~25us under x-stream contention), junk
    fillers at the join points so the PE never idles into a HAM
    re-throttle. More real matmuls complete before x is resident
    (~29.3us), which directly shortens the tail of the matmul stream.
  - o3's slab is prefetched (wpool bufs=4) so the steady state starts
    the moment the prologue drains.
  - The final o-tile's output DMA is split into partition halves on
    sync + scalar (both idle by then; 2KB rows stay at line rate).

PE roofline: 1024 MM x 512 cols @ ~2.4 GHz + issue = ~221 us.
"""

import os
import sys

for _p in ("/opt/trn_rl_repo", "/opt/pypackages"):
    if _p not in sys.path and os.path.isdir(_p):
        sys.path.append(_p)

import numpy as np
import ml_dtypes

import concourse.bass as bass
import concourse.tile as tile
from concourse import bacc, mybir
from concourse.bass_utils import run_bass_kernel_spmd

N_CORES = 8
R = 8                          # batch groups
BATCH = 4096
IN_F = 4096
OUT_F = 4096
P = 128
BR = BATCH // R                # 512 batch rows per core
KT = IN_F // P                 # 32 contraction tiles
OT = OUT_F // P                # 32 output-feature tiles per core
# x chunk schedule in k-tiles (1 k-tile = 128KB here)
XCHUNKS = [(0, 2), (2, 2), (4, 4), (8, 8), (16, 8), (24, 8)]
NPRE = 3                       # o-tiles in the staggered prologue

_F32 = mybir.dt.float32
_BF16 = mybir.dt.bfloat16
_BF16_NP = ml_dtypes.bfloat16

_compiled_nc = None


def _build():
    nc = bacc.Bacc("TRN2", target_bir_lowering=False, debug=False,
                   num_devices=N_CORES)

    # Host-pretiled layouts (see _prep_inputs):
    #   xt[p, it*BR + b]            = x_r[b, it*128 + p]            (bf16)
    #   wt[ot*128 + p, it*128 + o2] = w[ot*128 + o2, it*128 + p]    (bf16)
    #   bias_t[p, ot]               = b[ot*128 + p]                 (f32)
    xt = nc.dram_tensor("xt", [P, KT * BR], _BF16, kind="ExternalInput")
    wt = nc.dram_tensor("wt", [OT * P, KT * P], _BF16, kind="ExternalInput")
    bias = nc.dram_tensor("bias", [P, OT], _F32, kind="ExternalInput")
    out = nc.dram_tensor("out", [OUT_F, BR], _F32, kind="ExternalOutput")

    with tile.TileContext(nc) as tc:
        with tc.tile_pool(name="const", bufs=1) as const, \
             tc.tile_pool(name="wslab", bufs=4) as wpool, \
             tc.tile_pool(name="psum", bufs=6, space="PSUM") as pspool, \
             tc.tile_pool(name="yout", bufs=3) as ypool:

            bias_sb = const.tile([P, OT], _F32)
            nc.scalar.dma_start(out=bias_sb[:], in_=bias[:, :])

            # PE warm-up fuel with no DMA dependency: matmuls on it start
            # right after the engine preamble (~7us) and release the HAM
            # clock throttle before real work arrives.
            dummy = const.tile([P, P + BR], _BF16)
            nc.vector.memset(dummy[:], 1.0)

            ps_junk = pspool.tile([P, BR], _F32, name="psjunk", tag="ps")

            def junk_mms(n):
                for i in range(n):
                    nc.tensor.matmul(ps_junk[:], lhsT=dummy[:, 0:P],
                                     rhs=dummy[:, P:P + BR],
                                     start=(i == 0), stop=(i == n - 1))

            # ---- W slabs for the prologue o-tiles + o3 prefetch (sync)
            w_pre = []
            for ot in range(NPRE + 1):
                w_sb = wpool.tile([P, KT * P], _BF16, name=f"w{ot}", tag="w")
                nc.sync.dma_start(out=w_sb[:],
                                  in_=wt[ot * P:(ot + 1) * P, :])
                w_pre.append(w_sb)

            # ---- x on the gpsimd (SWDGE) ring, k-ascending
            x_sb = const.tile([P, KT * BR], _BF16)
            for it0, nit in XCHUNKS:
                nc.gpsimd.dma_start(
                    out=x_sb[:, it0 * BR:(it0 + nit) * BR],
                    in_=xt[:, it0 * BR:(it0 + nit) * BR])

            ps_pre = [pspool.tile([P, BR], _F32, name=f"psp{ot}", tag="ps")
                      for ot in range(NPRE + 1)]

            def mm(ot, it, ps=None, w=None):
                ps = ps if ps is not None else ps_pre[ot]
                w = w if w is not None else w_pre[ot]
                nc.tensor.matmul(
                    ps[:],
                    lhsT=w[:, it * P:(it + 1) * P],
                    rhs=x_sb[:, it * BR:(it + 1) * BR],
                    start=(it == 0), stop=(it == KT - 1))

            def evict(ot, ps, split=False):
                y_sb = ypool.tile([P, BR], _F32, name=f"y{ot}", tag="y")
                nc.scalar.activation(y_sb[:], ps[:],
                                     mybir.ActivationFunctionType.Identity,
                                     bias=bias_sb[:, ot:ot + 1])
                if split:
                    HP = P // 2
                    nc.sync.dma_start(out=out[ot * P:ot * P + HP, :],
                                      in_=y_sb[0:HP, :])
                    nc.scalar.dma_start(out=out[ot * P + HP:(ot + 1) * P, :],
                                        in_=y_sb[HP:P, :])
                else:
                    nc.scalar.dma_start(out=out[ot * P:(ot + 1) * P, :],
                                        in_=y_sb[:])

            # ---- prologue: o0 rides the x stream; o1 joins ~18us, o2
            # ~25us (slab arrival), each catching up on resident k-tiles.
            # Junk fillers absorb delivery jitter at the join points.
            junk_mms(12)                     # ~7.0 -> 12.2us, warms HAM
            mm(0, 0); mm(0, 1)               # x chunk0 ~12.5, W0 ~12.4
            junk_mms(2)
            mm(0, 2); mm(0, 3)               # ~13.5
            junk_mms(2)
            for it in range(4, 8): mm(0, it)     # ~15.6
            junk_mms(3)                      # bridge to o1's slab ~18
            for it in range(0, 4): mm(1, it)
            for it in range(8, 12): mm(0, it)    # x k8-15 ~19.9
            for it in range(4, 8): mm(1, it)
            for it in range(12, 16): mm(0, it)
            for it in range(8, 16): mm(1, it)
            junk_mms(2)                      # bridge to o2's slab ~25
            for it in range(0, 8): mm(2, it)
            for it in range(16, 24): mm(0, it)   # x k16-23 ~24.5
            for it in range(16, 24): mm(1, it)
            for it in range(8, 16): mm(2, it)
            for it in range(24, 32): mm(0, it)   # x k24-31 ~29.3
            evict(0, ps_pre[0])
            for it in range(24, 32): mm(1, it)
            evict(1, ps_pre[1])
            for it in range(16, 32): mm(2, it)
            evict(2, ps_pre[2])

            # o3: slab prefetched above
            for it in range(KT):
                mm(3, it)
            evict(3, ps_pre[3])

            # ---- steady state over o-tiles 4..31
            for ot in range(NPRE + 1, OT):
                w_sb = wpool.tile([P, KT * P], _BF16, name=f"w{ot}", tag="w")
                nc.sync.dma_start(out=w_sb[:],
                                  in_=wt[ot * P:(ot + 1) * P, :])
                ps = pspool.tile([P, BR], _F32, name=f"ps{ot}", tag="ps")
                for it in range(KT):
                    mm(ot, it, ps=ps, w=w_sb)
                evict(ot, ps, split=(ot == OT - 1))

    nc.compile()
    return nc


def _get_nc():
    global _compiled_nc
    if _compiled_nc is None:
        _compiled_nc = _build()
    return _compiled_nc


def _prep_inputs(inputs):
    x = np.ascontiguousarray(np.asarray(inputs["x"], dtype=np.float32))
    w = np.ascontiguousarray(np.asarray(inputs["weight"], dtype=np.float32))
    b = np.ascontiguousarray(np.asarray(inputs["bias"], dtype=np.float32))

    # x tiles per batch group r: [p, it*BR + b] = x_r[b, it*128 + p]
    xts = []
    for r in range(R):
        xs = x[r * BR:(r + 1) * BR, :]                      # [BR, IN_F]
        xt = xs.T.reshape(KT, P, BR).transpose(1, 0, 2)     # [P, KT, BR]
        xts.append(np.ascontiguousarray(
            xt.astype(_BF16_NP).reshape(P, KT * BR)))

    # W tiles: [ot*128 + p, it*128 + o2] = w[ot*128 + o2, it*128 + p]
    wtt = w.T.reshape(KT, P, OT, P).transpose(2, 1, 0, 3)   # [OT,P,KT,P]
    wt = np.ascontiguousarray(wtt.astype(_BF16_NP).reshape(OT * P, KT * P))
    bias_t = np.ascontiguousarray(b.reshape(OT, P).T)       # [P, OT]

    return [{"xt": xts[r], "wt": wt, "bias": bias_t} for r in range(R)]


def _run(inputs, trace=False, trace_cores=None):
    nc = _get_nc()
    in_maps = _prep_inputs(inputs)
    res = run_bass_kernel_spmd(nc, in_maps, core_ids=list(range(N_CORES)),
                               trace=trace, trace_cores=trace_cores)
    y = np.empty((BATCH, OUT_F), dtype=np.float32)
    for r in range(R):
        y[r * BR:(r + 1) * BR, :] = res.results[r]["out"].T
    return y, res


def kernel(**inputs):
    y, _ = _run(inputs)
    return y


# revision 7
# speedup vs baseline: 1.0219x; 1.0205x over previous
"""Trainium2 Bass kernel for y = x @ W^T + b  (4096x4096 @ 4096x4096 + 4096).

Sharding: data-parallel over batch, R=8 groups. Core r computes
yT_r = W @ x_r^T + b[:, None]  ([4096, 512], output transposed) and the
host reassembles y. No collectives.

All layout work happens on the host: x and W are transposed, tiled to
the exact SBUF layout, and cast to bf16 in numpy. The device kernel is
nothing but back-to-back bf16 matmuls (fp32 PSUM accumulate):

  - xT_r [128, 32*512] bf16 (4MB) resident in SBUF, DMA'd in graduated
    chunks (small first so compute starts early).
  - Prologue: the first 2 o-tiles' accumulations run chunk-major so the
    PE saturates while x is still arriving.
  - Steady state per o-tile (32): W slab [128, 32*128] bf16 DMA
    (triple-buffered, 147 GB/s sustained), 32 k-tile matmuls (N=512)
    accumulating in one PSUM bank, ScalarE eviction fused with bias
    add, HWDGE DMA out.

PE roofline: 1024 MM x 512 cols / 2.4 GHz = 218.5 us per core.
"""

import os
import sys

for _p in ("/opt/trn_rl_repo", "/opt/pypackages"):
    if _p not in sys.path and os.path.isdir(_p):
        sys.path.append(_p)

import numpy as np
import ml_dtypes

import concourse.bass as bass
import concourse.tile as tile
from concourse import bacc, mybir
from concourse.bass_utils import run_bass_kernel_spmd

N_CORES = 8
R = 8                          # batch groups
BATCH = 4096
IN_F = 4096
OUT_F = 4096
P = 128
BR = BATCH // R                # 512 batch rows per core
KT = IN_F // P                 # 32 contraction tiles
OT = OUT_F // P                # 32 output-feature tiles per core
# x chunk schedule in k-tiles (1 k-tile = 128KB here)
XCHUNKS = [(0, 2), (2, 2), (4, 4), (8, 8), (16, 8), (24, 8)]
NPRE = 2                       # o-tiles interleaved in the prologue

_F32 = mybir.dt.float32
_BF16 = mybir.dt.bfloat16
_BF16_NP = ml_dtypes.bfloat16

_compiled_nc = None


def _build():
    nc = bacc.Bacc("TRN2", target_bir_lowering=False, debug=False,
                   num_devices=N_CORES)

    # Host-pretiled layouts (see _prep_inputs):
    #   xt[p, it*BR + b]            = x_r[b, it*128 + p]            (bf16)
    #   wt[ot*128 + p, it*128 + o2] = w[ot*128 + o2, it*128 + p]    (bf16)
    #   bias_t[p, ot]               = b[ot*128 + p]                 (f32)
    xt = nc.dram_tensor("xt", [P, KT * BR], _BF16, kind="ExternalInput")
    wt = nc.dram_tensor("wt", [OT * P, KT * P], _BF16, kind="ExternalInput")
    bias = nc.dram_tensor("bias", [P, OT], _F32, kind="ExternalInput")
    out = nc.dram_tensor("out", [OUT_F, BR], _F32, kind="ExternalOutput")

    with tile.TileContext(nc) as tc:
        with tc.tile_pool(name="const", bufs=1) as const, \
             tc.tile_pool(name="wslab", bufs=3) as wpool, \
             tc.tile_pool(name="psum", bufs=4, space="PSUM") as pspool, \
             tc.tile_pool(name="yout", bufs=3) as ypool:

            bias_sb = const.tile([P, OT], _F32)
            nc.scalar.dma_start(out=bias_sb[:], in_=bias[:, :])

            # PE warm-up fuel: a junk tile whose first 16 cols come from the
            # bias DMA, so matmuls on it cannot start before ~10us -- after
            # first_useful, inside the PE-idle window before chunk0 lands.
            dummy = const.tile([P, 528], _BF16)
            nc.vector.memset(dummy[:, 16:], 1.0)
            nc.vector.tensor_copy(out=dummy[:, :16], in_=bias_sb[:, :16])

            ps_junk = pspool.tile([P, BR], _F32, name="psjunk", tag="ps")

            def junk_mms(n):
                for i in range(n):
                    nc.tensor.matmul(ps_junk[:], lhsT=dummy[:, 0:P],
                                     rhs=dummy[:, 16:16 + BR],
                                     start=(i == 0), stop=(i == n - 1))

            # ---- W slabs for the prologue o-tiles
            w_pre = []
            for ot in range(NPRE):
                w_sb = wpool.tile([P, KT * P], _BF16, name=f"w{ot}", tag="w")
                nc.sync.dma_start(out=w_sb[:],
                                  in_=wt[ot * P:(ot + 1) * P, :])
                w_pre.append(w_sb)

            # ---- x on the gpsimd (SWDGE) ring
            x_sb = const.tile([P, KT * BR], _BF16)
            for it0, nit in XCHUNKS:
                nc.gpsimd.dma_start(
                    out=x_sb[:, it0 * BR:(it0 + nit) * BR],
                    in_=xt[:, it0 * BR:(it0 + nit) * BR])

            # ~3.5us of junk matmuls: release the HAM clock throttle while
            # the PE would otherwise idle waiting for the first x chunk.
            junk_mms(8)

            def evict(ot, y_sb, ps):
                nc.scalar.activation(y_sb[:], ps[:],
                                     mybir.ActivationFunctionType.Identity,
                                     bias=bias_sb[:, ot:ot + 1])
                nc.scalar.dma_start(out=out[ot * P:(ot + 1) * P, :],
                                    in_=y_sb[:])

            # ---- prologue: first NPRE o-tiles chunk-major over x arrival.
            # A few junk matmuls after the small early chunks bridge the
            # delivery gaps so the warm clock never re-throttles.
            ps_pre = [pspool.tile([P, BR], _F32, name=f"psp{ot}", tag="ps")
                      for ot in range(NPRE)]
            for ci, (it0, nit) in enumerate(XCHUNKS):
                for ot in range(NPRE):
                    for l in range(nit):
                        it = it0 + l
                        nc.tensor.matmul(
                            ps_pre[ot][:],
                            lhsT=w_pre[ot][:, it * P:(it + 1) * P],
                            rhs=x_sb[:, it * BR:(it + 1) * BR],
                            start=(it == 0), stop=(it == KT - 1))
                if ci in (0, 1, 2):
                    junk_mms(4)
            for ot in range(NPRE):
                y_sb = ypool.tile([P, BR], _F32, name=f"y{ot}", tag="y")
                evict(ot, y_sb, ps_pre[ot])

            # ---- steady state over the remaining o-tiles
            for ot in range(NPRE, OT):
                w_sb = wpool.tile([P, KT * P], _BF16, name=f"w{ot}", tag="w")
                nc.sync.dma_start(out=w_sb[:],
                                  in_=wt[ot * P:(ot + 1) * P, :])

                y_sb = ypool.tile([P, BR], _F32, name=f"y{ot}", tag="y")
                ps = pspool.tile([P, BR], _F32, name=f"ps{ot}", tag="ps")
                for it in range(KT):
                    nc.tensor.matmul(ps[:],
                                     lhsT=w_sb[:, it * P:(it + 1) * P],
                                     rhs=x_sb[:, it * BR:(it + 1) * BR],
                                     start=(it == 0), stop=(it == KT - 1))
                evict(ot, y_sb, ps)

    nc.compile()
    return nc


def _get_nc():
    global _compiled_nc
    if _compiled_nc is None:
        _compiled_nc = _build()
    return _compiled_nc


def _prep_inputs(inputs):
    x = np.ascontiguousarray(np.asarray(inputs["x"], dtype=np.float32))
    w = np.ascontiguousarray(np.asarray(inputs["weight"], dtype=np.float32))
    b = np.ascontiguousarray(np.asarray(inputs["bias"], dtype=np.float32))

    # x tiles per batch group r: [p, it*BR + b] = x_r[b, it*128 + p]
    xts = []
    for r in range(R):
        xs = x[r * BR:(r + 1) * BR, :]                      # [BR, IN_F]
        xt = xs.T.reshape(KT, P, BR).transpose(1, 0, 2)     # [P, KT, BR]
        xts.append(np.ascontiguousarray(
            xt.astype(_BF16_NP).reshape(P, KT * BR)))

    # W tiles: [ot*128 + p, it*128 + o2] = w[ot*128 + o2, it*128 + p]
    wtt = w.T.reshape(KT, P, OT, P).transpose(2, 1, 0, 3)   # [OT,P,KT,P]
    wt = np.ascontiguousarray(wtt.astype(_BF16_NP).reshape(OT * P, KT * P))
    bias_t = np.ascontiguousarray(b.reshape(OT, P).T)       # [P, OT]

    return [{"xt": xts[r], "wt": wt, "bias": bias_t} for r in range(R)]


def _run(inputs, trace=False, trace_cores=None):
    nc = _get_nc()
    in_maps = _prep_inputs(inputs)
    res = run_bass_kernel_spmd(nc, in_maps, core_ids=list(range(N_CORES)),
                               trace=trace, trace_cores=trace_cores)
    y = np.empty((BATCH, OUT_F), dtype=np.float32)
    for r in range(R):
        y[r * BR:(r + 1) * BR, :] = res.results[r]["out"].T
    return y, res


def kernel(**inputs):
    y, _ = _run(inputs)
    return y

